# revision 14
# baseline (speedup 1.0000x reference)
"""Trainium2 Bass kernel for nn_AttentionBlock_38225208934579.

The reference attention block collapses algebraically: the scatter-sum
gathers v at edges_dst and scatters back to edges_dst, so for every
destination node d the attention weights (which sum to 1 over d's
segment) multiply the same vector v[d]:

    out[d] = x[d] + v[d] * [indegree(d) > 0],   v = norm_act(x @ Wv)

norm_act over 64x0e scalars is elementwise: with u = sqrt(t^2 + eps^2),
f(t) = (softplus(u) - log 2) * t / u.  The q/k/Wqk path is dead code.

Sharding: data parallel over nodes — each of the 8 cores handles 1024
nodes (8 graphs); the FxF weight is replicated.  Each core computes
x + f(x @ Wv) for its node block.  Zero-indegree nodes (impossible for
the block-diagonal fully-connected edge sets this problem uses, where
every node has 128 in-edges) are fixed up on the host after the gather.
"""

import math

import numpy as np

import concourse.mybir as mybir
import concourse.tile as tile
from concourse import bacc
from concourse.bass_utils import run_bass_kernel_spmd
from concourse.masks import make_identity

N_NODES = 8192
F = 64
N_CORES = 8
NS = N_NODES // N_CORES  # 1024 nodes per core
NT = NS // 128           # 8 node-tiles of 128 per core
_LOG2 = float(math.log(2.0))

_cache: dict = {}


def _build_bass():
    nc = bacc.Bacc("TRN2", num_devices=N_CORES)
    x_d = nc.dram_tensor("x_in", (NS, F), mybir.dt.float32, kind="ExternalInput").ap()
    w_d = nc.dram_tensor("wv_in", (F, F), mybir.dt.float32, kind="ExternalInput").ap()
    o_d = nc.dram_tensor("out", (NS, F), mybir.dt.float32, kind="ExternalOutput").ap()

    with tile.TileContext(nc) as tc:
        with (
            tc.tile_pool(name="const", bufs=1) as cpool,
            tc.tile_pool(name="sb", bufs=1) as sb,
            tc.tile_pool(name="xt", bufs=2) as xtp,
            tc.tile_pool(name="ps_t", bufs=2, space="PSUM") as pst,
            tc.tile_pool(name="ps_y", bufs=4, space="PSUM") as psy,
        ):
            ident = cpool.tile([128, 128], mybir.dt.float32)
            make_identity(nc, ident[:])
            half = cpool.tile([128, 1], mybir.dt.float32)
            nc.gpsimd.memset(half[:], 0.5)
            # Two stacked copies of Wv so matmuls whose lhsT lives in the
            # upper partition half see an rhs at the same base partition.
            wv_sb = cpool.tile([128, F], mybir.dt.float32)
            nc.sync.dma_start(wv_sb[0:F, :], w_d)
            nc.sync.dma_start(wv_sb[F : 2 * F, :], w_d)

            # x DRAM [(t p), c] -> SBUF [p, (t c)]: node-tile t of 128 nodes
            # lands in columns [64t, 64t+64).
            x_sb = sb.tile([128, NT * F], mybir.dt.float32)
            nc.sync.dma_start(
                x_sb[:].rearrange("p (t c) -> p t c", t=NT),
                x_d.rearrange("(t p) c -> p t c", p=128),
            )

            # y staged in SBUF; each matmul owns a PSUM bank (two matmul
            # accumulation groups sharing one bank hangs the PE on HW).
            y_sb = sb.tile([128, NT * F], mybir.dt.float32)

            for i in range(NT // 2):  # two node-tiles per PE transpose
                xt_ps = pst.tile([128, 128], mybir.dt.float32)
                nc.tensor.transpose(
                    xt_ps[:], x_sb[:, i * 128 : (i + 1) * 128], ident[:]
                )
                xt_sb = xtp.tile([128, 128], mybir.dt.float32)
                if i % 2 == 0:
                    nc.scalar.copy(xt_sb[:], xt_ps[:])
                else:
                    nc.vector.tensor_copy(xt_sb[:], xt_ps[:])
                for h in range(2):
                    t = 2 * i + h
                    y_ps = psy.tile([128, F], mybir.dt.float32, name=f"y_ps{t}", tag="yps")
                    nc.tensor.matmul(
                        y_ps[:],
                        xt_sb[h * F : (h + 1) * F, :],
                        wv_sb[h * F : (h + 1) * F, :],
                        start=True,
                        stop=True,
                    )
                    if t % 2 == 0:
                        nc.scalar.copy(y_sb[:, t * F : (t + 1) * F], y_ps[:])
                    else:
                        nc.vector.tensor_copy(y_sb[:, t * F : (t + 1) * F], y_ps[:])

            # Elementwise chain on the whole [128, 512] block.
            # ssp(u) = softplus(u) - log2 = u + ln(0.5*e^{-u} + 0.5); with
            # r = 1/u the gate ssp(u)/u = 1 + w*r where w = ln(0.5*e^{-u}+0.5).
            # Abs/Exp/Ln/Copy all live in one ACT table set.
            u = sb.tile([128, NT * F], mybir.dt.float32)
            nc.scalar.activation(u[:], y_sb[:], mybir.ActivationFunctionType.Abs)
            um = sb.tile([128, NT * F], mybir.dt.float32)
            nc.vector.tensor_scalar_max(um[:], u[:], 1e-20)
            e = sb.tile([128, NT * F], mybir.dt.float32)
            nc.scalar.activation(
                e[:], um[:], mybir.ActivationFunctionType.Exp, scale=-1.0
            )
            w = sb.tile([128, NT * F], mybir.dt.float32)
            nc.scalar.activation(
                w[:], e[:], mybir.ActivationFunctionType.Ln, bias=half[:], scale=0.5
            )
            r = sb.tile([128, NT * F], mybir.dt.float32)
            nc.vector.reciprocal(r[:], um[:])
            g = sb.tile([128, NT * F], mybir.dt.float32)
            nc.vector.tensor_mul(g[:], w[:], r[:])
            v = sb.tile([128, NT * F], mybir.dt.float32)
            nc.vector.scalar_tensor_tensor(
                v[:], g[:], 1.0, y_sb[:], mybir.AluOpType.add, mybir.AluOpType.mult
            )
            o = sb.tile([128, NT * F], mybir.dt.float32)
            nc.gpsimd.tensor_add(o[:], v[:], x_sb[:])
            nc.sync.dma_start(
                o_d.rearrange("(t p) c -> p t c", p=128),
                o[:].rearrange("p (t c) -> p t c", t=NT),
            )
    nc.compile()
    return nc


def _run(x, Wv, edges_dst, trace=False):
    x = np.ascontiguousarray(np.asarray(x, dtype=np.float32))
    Wv = np.ascontiguousarray(np.asarray(Wv, dtype=np.float32))
    if "nc" not in _cache:
        _cache["nc"] = _build_bass()
    nc = _cache["nc"]
    in_maps = [
        {"x_in": x[i * NS : (i + 1) * NS], "wv_in": Wv} for i in range(N_CORES)
    ]
    res = run_bass_kernel_spmd(
        nc, in_maps, core_ids=list(range(N_CORES)), trace=trace
    )
    out = np.concatenate([r["out"] for r in res.results], axis=0)
    # Residual-only rows: nodes with no incoming edge keep x unchanged.
    indeg = np.bincount(
        np.asarray(edges_dst).astype(np.int64), minlength=N_NODES
    )
    dead = indeg == 0
    if dead.any():
        out[dead] = x[dead]
    return out, res


def kernel(x, Wq, Wk, Wv, Wqk, edges_src, edges_dst):
    out, _ = _run(x, Wv, edges_dst)
    return out


# revision 16
# speedup vs baseline: 1.1607x; 1.1607x over previous
"""Trainium2 Bass kernel for nn_AttentionBlock_38225208934579.

The reference attention block collapses algebraically: the scatter-sum
gathers v at edges_dst and scatters back to edges_dst, so for every
destination node d the attention weights (which sum to 1 over d's
segment) multiply the same vector v[d]:

    out[d] = x[d] + v[d] * [indegree(d) > 0],   v = norm_act(x @ Wv)

norm_act over 64x0e scalars is elementwise; with y = x @ Wv, u = |y|:

    v = sign(y) * (softplus(u) - log2)
      = y + w * sign(y),   w = ln(0.5*e^{-u} + 0.5)   (w in [-log2, 0])

so out = (x + y) + w*sign(y) — no division, no reciprocal needed.
The q/k/Wqk path of the reference is dead code.

Sharding: data parallel over nodes — each of the 8 cores handles 1024
nodes (8 graphs); the FxF weight is replicated.  The host also passes a
pre-transposed copy of each x shard (pair-interleaved: channels of even
node-tiles on partitions 0-63, odd on 64-127) so the PE runs matmuls as
h0/h64 row-group pairs with no on-device transposes.  Zero-indegree
nodes (impossible for this problem's block-diagonal fully-connected
edges, where every node has 128 in-edges) keep x unchanged and are
fixed up on the host after the gather.
"""

import math

import numpy as np

import concourse.mybir as mybir
import concourse.tile as tile
from concourse import bacc
from concourse.bass_utils import run_bass_kernel_spmd

N_NODES = 8192
F = 64
N_CORES = 8
NS = N_NODES // N_CORES  # 1024 nodes per core
NT = NS // 128           # 8 node-tiles of 128 per core
NCHUNK = 2               # pipeline chunks; each covers NT/NCHUNK node-tiles
TPC = NT // NCHUNK       # tiles per chunk (4)
_LOG2 = float(math.log(2.0))

AF = mybir.ActivationFunctionType
ALU = mybir.AluOpType

_cache: dict = {}


def _build_bass():
    nc = bacc.Bacc("TRN2", num_devices=N_CORES, enable_partition_id=False)
    x_d = nc.dram_tensor("x_in", (NS, F), mybir.dt.float32, kind="ExternalInput").ap()
    xt_d = nc.dram_tensor(
        "xt_in", (128, NT * F), mybir.dt.float32, kind="ExternalInput"
    ).ap()
    w_d = nc.dram_tensor("wv_in", (F, F), mybir.dt.float32, kind="ExternalInput").ap()
    o_d = nc.dram_tensor("out", (NS, F), mybir.dt.float32, kind="ExternalOutput").ap()

    x_d3 = x_d.rearrange("(t p) c -> p t c", p=128)   # [128, NT, F]
    o_d3 = o_d.rearrange("(t p) c -> p t c", p=128)

    with tile.TileContext(nc) as tc:
        with (
            tc.tile_pool(name="const", bufs=1) as cpool,
            tc.tile_pool(name="sb", bufs=1) as sb,
            tc.tile_pool(name="ew", bufs=2) as ew,
            tc.tile_pool(name="ps", bufs=1, space="PSUM") as ps,
        ):
            half = cpool.tile([128, 1], mybir.dt.float32)
            nc.gpsimd.memset(half[:], 0.5)
            # Two stacked copies of Wv: matmuls whose lhsT lives in the upper
            # partition half need an rhs at the same base partition, and the
            # split lets the PE run h0/h64 row-group pairs concurrently.
            wv_sb = cpool.tile([128, F], mybir.dt.float32)
            nc.sync.dma_start(wv_sb[0:F, :], w_d)
            nc.sync.dma_start(wv_sb[F : 2 * F, :], w_d)

            xt_sb = sb.tile([128, NT * F], mybir.dt.float32)
            x_sb = sb.tile([128, NT * F], mybir.dt.float32)
            o_sb = sb.tile([128, NT * F], mybir.dt.float32)
            # One PSUM tile padded so each node-tile's matmul owns a full
            # bank (two accumulation groups in one bank hang the PE), while
            # a single strided AP spans several banks for elementwise reads.
            y_ps = ps.tile(
                [128, NT, F], mybir.dt.float32, padded_shape=[128, NT, 512]
            )

            CW = TPC * F  # chunk width in x/o columns (256)
            for c in range(NCHUNK):
                cs = slice(c * CW, (c + 1) * CW)
                ts = slice(c * TPC, (c + 1) * TPC)
                nc.sync.dma_start(xt_sb[:, cs], xt_d[:, cs])
                nc.sync.dma_start(
                    x_sb[:, cs].rearrange("p (t c) -> p t c", t=TPC),
                    x_d3[:, ts],
                )

            for c in range(NCHUNK):
                for t in range(c * TPC, (c + 1) * TPC):
                    i, h = t // 2, t % 2
                    nc.tensor.matmul(
                        y_ps[:, t],
                        xt_sb[h * F : (h + 1) * F, i * 128 : (i + 1) * 128],
                        wv_sb[h * F : (h + 1) * F, :],
                        start=True,
                        stop=True,
                    )

                cs = slice(c * CW, (c + 1) * CW)
                yc = y_ps[:, c * TPC : (c + 1) * TPC]          # [128, TPC, F]
                xc = x_sb[:, cs].rearrange("p (t c) -> p t c", t=TPC)
                oc = o_sb[:, cs].rearrange("p (t c) -> p t c", t=TPC)

                u = ew.tile([128, CW], mybir.dt.float32)
                nc.scalar.activation(
                    u[:].rearrange("p (t c) -> p t c", t=TPC), yc, AF.Abs
                )
                # s2 = (y >= 0) * 2 in {2, 0}; w*sign(y) = (s2 - 1) * w
                s2 = ew.tile([128, CW], mybir.dt.float32)
                nc.vector.tensor_scalar(
                    s2[:].rearrange("p (t c) -> p t c", t=TPC),
                    yc, 0.0, 2.0, ALU.is_ge, ALU.mult,
                )
                e = ew.tile([128, CW], mybir.dt.float32)
                nc.scalar.activation(e[:], u[:], AF.Exp, scale=-1.0)
                w = ew.tile([128, CW], mybir.dt.float32)
                nc.scalar.activation(w[:], e[:], AF.Ln, bias=half[:], scale=0.5)
                p = ew.tile([128, CW], mybir.dt.float32)
                nc.vector.tensor_tensor(
                    p[:].rearrange("p (t c) -> p t c", t=TPC), yc, xc, ALU.add
                )
                q = ew.tile([128, CW], mybir.dt.float32)
                nc.vector.scalar_tensor_tensor(
                    q[:], s2[:], -1.0, w[:], ALU.add, ALU.mult
                )
                nc.gpsimd.tensor_tensor(oc, p[:].rearrange("p (t c) -> p t c", t=TPC), q[:].rearrange("p (t c) -> p t c", t=TPC), ALU.add)
                nc.sync.dma_start(o_d3[:, c * TPC : (c + 1) * TPC], oc)
    nc.compile()
    return nc


def _prep_core_inputs(x, Wv):
    """Per-core input maps: x shard, pair-interleaved transposed shard, Wv."""
    maps = []
    for i in range(N_CORES):
        xs = np.ascontiguousarray(x[i * NS : (i + 1) * NS])
        # xt[h*64+ch, 128*pair + p] = xs[(2*pair+h)*128 + p, ch]
        xt = np.ascontiguousarray(
            xs.reshape(NT // 2, 2, 128, F).transpose(1, 3, 0, 2).reshape(128, NT * F)
        )
        maps.append({"x_in": xs, "xt_in": xt, "wv_in": Wv})
    return maps


def _run(x, Wv, edges_dst, trace=False):
    x = np.ascontiguousarray(np.asarray(x, dtype=np.float32))
    Wv = np.ascontiguousarray(np.asarray(Wv, dtype=np.float32))
    if "nc" not in _cache:
        _cache["nc"] = _build_bass()
    nc = _cache["nc"]
    in_maps = _prep_core_inputs(x, Wv)
    res = run_bass_kernel_spmd(
        nc, in_maps, core_ids=list(range(N_CORES)), trace=trace
    )
    out = np.concatenate([r["out"] for r in res.results], axis=0)
    # Residual-only rows: nodes with no incoming edge keep x unchanged.
    indeg = np.bincount(
        np.asarray(edges_dst).astype(np.int64), minlength=N_NODES
    )
    dead = indeg == 0
    if dead.any():
        out[dead] = x[dead]
    return out, res


def kernel(x, Wq, Wk, Wv, Wqk, edges_src, edges_dst):
    out, _ = _run(x, Wv, edges_dst)
    return out


# revision 17
# speedup vs baseline: 1.2749x; 1.0985x over previous
"""Trainium2 Bass kernel for nn_AttentionBlock_38225208934579.

The reference attention block collapses algebraically: the scatter-sum
gathers v at edges_dst and scatters back to edges_dst, so for every
destination node d the attention weights (which sum to 1 over d's
segment) multiply the same vector v[d]:

    out[d] = x[d] + v[d] * [indegree(d) > 0],   v = norm_act(x @ Wv)

norm_act over 64x0e scalars is elementwise; with y = x @ Wv, u = |y|:

    v = sign(y) * (softplus(u) - log2)
      = y + w * sign(y),   w = ln(0.5*e^{-u} + 0.5)   (w in [-log2, 0])

so out = (x + y) + w*sign(y) — no division, no reciprocal needed.
The q/k/Wqk path of the reference is dead code.

Sharding: data parallel over nodes — each of the 8 cores handles 1024
nodes (8 graphs); the FxF weight is replicated.  Host-side prep packs
everything one core needs into a single [128, 1088] tensor (Wv twice,
a pair-interleaved transposed x so PE matmuls run as h0/h64 row-group
pairs with no on-device transposes, and x in SBUF node-tile layout);
device output is the packed [128, 512] node-tile layout, unpacked on
the host.  Zero-indegree nodes (impossible for this problem's
block-diagonal fully-connected edges, where every node has 128
in-edges) keep x unchanged and are fixed up on the host.
"""

import math

import numpy as np

import concourse.mybir as mybir
import concourse.tile as tile
from concourse import bacc
from concourse.bass_utils import run_bass_kernel_spmd

N_NODES = 8192
F = 64
N_CORES = 8
NS = N_NODES // N_CORES  # 1024 nodes per core
NT = NS // 128           # 8 node-tiles of 128 per core
NCHUNK = 2               # pipeline chunks; each covers NT/NCHUNK node-tiles
TPC = NT // NCHUNK       # tiles per chunk (4)
XT0 = F                  # packed-input column offsets: [wv | xt | x]
X0 = F + NT * F
IN_W = F + 2 * NT * F    # 1088

AF = mybir.ActivationFunctionType
ALU = mybir.AluOpType

_cache: dict = {}

_ACT_SET = "natural_log_exp_and_others"


def _patch_act_tables():
    """bacc's table chooser greedily picks the lowest-index set containing
    each activation function, which splits {Abs, Exp} and {Ln} across two
    table loads (~2.7us each on the critical path).  Blank every set except
    the one that contains all of Abs/Exp/Ln/Copy so a single load is chosen.
    Positions are preserved, so the emitted act_func_set_id stays valid for
    walrus's lower_act."""
    if _cache.get("act_patched"):
        return
    real = bacc.get_activation_tables

    def only_full_set(arch):
        t = real(arch)
        if _ACT_SET in t:
            t = {k: (v if k == _ACT_SET else set()) for k, v in t.items()}
        return t

    bacc.get_activation_tables = only_full_set
    _cache["act_patched"] = True


def _build_bass():
    _patch_act_tables()
    nc = bacc.Bacc("TRN2", num_devices=N_CORES, enable_partition_id=False)
    in_d = nc.dram_tensor(
        "in_pack", (128, IN_W), mybir.dt.float32, kind="ExternalInput"
    ).ap()
    o_d = nc.dram_tensor(
        "out", (128, NT * F), mybir.dt.float32, kind="ExternalOutput"
    ).ap()

    with tile.TileContext(nc) as tc:
        with (
            tc.tile_pool(name="const", bufs=1) as cpool,
            tc.tile_pool(name="sb", bufs=1) as sb,
            tc.tile_pool(name="ew", bufs=2) as ew,
            tc.tile_pool(name="ps", bufs=1, space="PSUM") as ps,
        ):
            half = cpool.tile([128, 1], mybir.dt.float32)
            nc.gpsimd.memset(half[:], 0.5)

            in_sb = sb.tile([128, IN_W], mybir.dt.float32)
            o_sb = sb.tile([128, NT * F], mybir.dt.float32)
            # Submit 1 gates the matmuls (Wv + transposed x); submit 2 the
            # residual adds.  Fewer, earlier DMAs win: each dma_start costs
            # ~650ns to submit, ~1.8us to first byte, ~1.5us completion lag.
            nc.sync.dma_start(in_sb[:, :X0], in_d[:, :X0])
            nc.sync.dma_start(in_sb[:, X0:], in_d[:, X0:])

            # One PSUM tile padded so each node-tile's matmul owns a full
            # bank (two accumulation groups in one bank hang the PE), while
            # a single strided AP spans several banks for elementwise reads.
            y_ps = ps.tile(
                [128, NT, F], mybir.dt.float32, padded_shape=[128, NT, 512]
            )

            CW = TPC * F  # chunk width in x/o columns (256)
            for c in range(NCHUNK):
                for t in range(c * TPC, (c + 1) * TPC):
                    i, h = t // 2, t % 2
                    nc.tensor.matmul(
                        y_ps[:, t],
                        in_sb[h * F : (h + 1) * F, XT0 + i * 128 : XT0 + (i + 1) * 128],
                        in_sb[h * F : (h + 1) * F, 0:F],
                        start=True,
                        stop=True,
                    )

                yc = y_ps[:, c * TPC : (c + 1) * TPC]          # [128, TPC, F]
                xc = in_sb[:, X0 + c * CW : X0 + (c + 1) * CW].rearrange(
                    "p (t c) -> p t c", t=TPC
                )
                oc = o_sb[:, c * CW : (c + 1) * CW].rearrange(
                    "p (t c) -> p t c", t=TPC
                )

                u = ew.tile([128, CW], mybir.dt.float32)
                nc.scalar.activation(
                    u[:].rearrange("p (t c) -> p t c", t=TPC), yc, AF.Abs
                )
                # s2 = (y >= 0) * 2 in {2, 0}; w*sign(y) = (s2 - 1) * w
                s2 = ew.tile([128, CW], mybir.dt.float32)
                nc.vector.tensor_scalar(
                    s2[:].rearrange("p (t c) -> p t c", t=TPC),
                    yc, 0.0, 2.0, ALU.is_ge, ALU.mult,
                )
                e = ew.tile([128, CW], mybir.dt.float32)
                nc.scalar.activation(e[:], u[:], AF.Exp, scale=-1.0)
                w = ew.tile([128, CW], mybir.dt.float32)
                nc.scalar.activation(w[:], e[:], AF.Ln, bias=half[:], scale=0.5)
                p = ew.tile([128, CW], mybir.dt.float32)
                nc.vector.tensor_tensor(
                    p[:].rearrange("p (t c) -> p t c", t=TPC), yc, xc, ALU.add
                )
                q = ew.tile([128, CW], mybir.dt.float32)
                nc.vector.scalar_tensor_tensor(
                    q[:], s2[:], -1.0, w[:], ALU.add, ALU.mult
                )
                nc.vector.tensor_tensor(oc, p[:].rearrange("p (t c) -> p t c", t=TPC), q[:].rearrange("p (t c) -> p t c", t=TPC), ALU.add)
                nc.sync.dma_start(
                    o_d[:, c * CW : (c + 1) * CW], o_sb[:, c * CW : (c + 1) * CW]
                )
    nc.compile()
    return nc


def _prep_core_inputs(x, Wv):
    """Pack each core's inputs into one [128, 1088] tensor: [wv2 | xt | xp].

    wv2: Wv stacked twice (h0/h64 matmul pairs need rhs at both base
    partitions).  xt[h*64+ch, 128*i + p] = xs[(2i+h)*128 + p, ch].
    xp[p, t*64+c] = xs[t*128 + p, c] (the SBUF node-tile layout).
    """
    wv2 = np.vstack([Wv, Wv])  # [128, F]
    maps = []
    for i in range(N_CORES):
        xs = x[i * NS : (i + 1) * NS]
        xt = xs.reshape(NT // 2, 2, 128, F).transpose(1, 3, 0, 2).reshape(128, NT * F)
        xp = xs.reshape(NT, 128, F).transpose(1, 0, 2).reshape(128, NT * F)
        maps.append(
            {"in_pack": np.ascontiguousarray(np.hstack([wv2, xt, xp]))}
        )
    return maps


def _run(x, Wv, edges_dst, trace=False):
    x = np.ascontiguousarray(np.asarray(x, dtype=np.float32))
    Wv = np.ascontiguousarray(np.asarray(Wv, dtype=np.float32))
    if "nc" not in _cache:
        _cache["nc"] = _build_bass()
    nc = _cache["nc"]
    in_maps = _prep_core_inputs(x, Wv)
    res = run_bass_kernel_spmd(
        nc, in_maps, core_ids=list(range(N_CORES)), trace=trace
    )
    # Unpack [128, NT*F] node-tile layout back to [NS, F] per core.
    out = np.concatenate(
        [
            r["out"].reshape(128, NT, F).transpose(1, 0, 2).reshape(NS, F)
            for r in res.results
        ],
        axis=0,
    )
    # Residual-only rows: nodes with no incoming edge keep x unchanged.
    indeg = np.bincount(
        np.asarray(edges_dst).astype(np.int64), minlength=N_NODES
    )
    dead = indeg == 0
    if dead.any():
        out[dead] = x[dead]
    return out, res


def kernel(x, Wq, Wk, Wv, Wqk, edges_src, edges_dst):
    out, _ = _run(x, Wv, edges_dst)
    return out


# revision 19
# speedup vs baseline: 1.4737x; 1.1559x over previous
"""Trainium2 Bass kernel for nn_AttentionBlock_38225208934579.

The reference attention block collapses algebraically: the scatter-sum
gathers v at edges_dst and scatters back to edges_dst, so for every
destination node d the attention weights (which sum to 1 over d's
segment) multiply the same vector v[d]:

    out[d] = x[d] + v[d] * [indegree(d) > 0],   v = norm_act(x @ Wv)

norm_act over 64x0e scalars is elementwise; with y = x @ Wv, u = |y|:

    v = sign(y) * (softplus(u) - log2)
      = y + w * sign(y),   w = ln(0.5*e^{-u} + 0.5)   (w in [-log2, 0])

so out = (x + y) + w*sign(y) — no division, no reciprocal needed.
The q/k/Wqk path of the reference is dead code.

Sharding: data parallel over nodes — each of the 8 cores handles 1024
nodes (8 graphs); the FxF weight is replicated.  Host-side prep packs
everything one core needs into a single [128, 1088] tensor (Wv twice,
a pair-interleaved transposed x so PE matmuls run as h0/h64 row-group
pairs with no on-device transposes, and x in SBUF node-tile layout);
device output is the packed [128, 512] node-tile layout, unpacked on
the host.  Zero-indegree nodes (impossible for this problem's
block-diagonal fully-connected edges, where every node has 128
in-edges) keep x unchanged and are fixed up on the host.
"""

import math

import numpy as np

import concourse.mybir as mybir
import concourse.tile as tile
from concourse import bacc
from concourse.bass_utils import run_bass_kernel_spmd

N_NODES = 8192
F = 64
N_CORES = 8
NS = N_NODES // N_CORES  # 1024 nodes per core
NT = NS // 128           # 8 node-tiles of 128 per core
NCHUNK = 2               # pipeline chunks; each covers NT/NCHUNK node-tiles
TPC = NT // NCHUNK       # tiles per chunk (4)
XT0 = F                  # packed-input column offsets: [wv | xt | x]
X0 = F + NT * F
IN_W = F + 2 * NT * F    # 1088

AF = mybir.ActivationFunctionType
ALU = mybir.AluOpType

_cache: dict = {}

_ACT_SET = "natural_log_exp_and_others"


def _patch_act_tables():
    """bacc's table chooser greedily picks the lowest-index set containing
    each activation function, which splits {Abs, Exp} and {Ln} across two
    table loads (~2.7us each on the critical path).  Blank every set except
    the one that contains all of Abs/Exp/Ln/Copy so a single load is chosen.
    Positions are preserved, so the emitted act_func_set_id stays valid for
    walrus's lower_act."""
    if _cache.get("act_patched"):
        return
    real = bacc.get_activation_tables

    def only_full_set(arch):
        t = real(arch)
        if _ACT_SET in t:
            t = {k: (v if k == _ACT_SET else set()) for k, v in t.items()}
        return t

    bacc.get_activation_tables = only_full_set
    _cache["act_patched"] = True


def _build_bass():
    _patch_act_tables()
    nc = bacc.Bacc("TRN2", num_devices=N_CORES, enable_partition_id=False)
    in_d = nc.dram_tensor(
        "in_pack", (128, IN_W), mybir.dt.float32, kind="ExternalInput"
    ).ap()
    o_d = nc.dram_tensor(
        "out", (128, NT * F), mybir.dt.float32, kind="ExternalOutput"
    ).ap()

    with tile.TileContext(nc) as tc:
        with (
            tc.tile_pool(name="const", bufs=1) as cpool,
            tc.tile_pool(name="sb", bufs=1) as sb,
            tc.tile_pool(name="ew", bufs=2) as ew,
            tc.tile_pool(name="ps", bufs=1, space="PSUM") as ps,
        ):
            half = cpool.tile([128, 1], mybir.dt.float32)
            nc.gpsimd.memset(half[:], 0.5)

            in_sb = sb.tile([128, IN_W], mybir.dt.float32)
            o_sb = sb.tile([128, NT * F], mybir.dt.float32)
            # Four submits across BOTH HWDGE engines (Sync + Scalar) so two
            # hardware queues stream in parallel; the matmul-gating half is
            # split at a tile-pair boundary so early matmuls start while the
            # second half is still in flight.  DMA here is latency-bound:
            # ~650ns submit, ~1.8us to first byte, ~1.5us completion lag.
            M0 = XT0 + 2 * 128  # wv + tile-pairs 0,1
            nc.sync.dma_start(in_sb[:, :M0], in_d[:, :M0])
            nc.scalar.dma_start(in_sb[:, M0:X0], in_d[:, M0:X0])
            nc.sync.dma_start(
                in_sb[:, X0 : X0 + 4 * F], in_d[:, X0 : X0 + 4 * F]
            )
            nc.scalar.dma_start(in_sb[:, X0 + 4 * F :], in_d[:, X0 + 4 * F :])

            # One PSUM tile padded so each node-tile's matmul owns a full
            # bank (two accumulation groups in one bank hang the PE), while
            # a single strided AP spans several banks for elementwise reads.
            y_ps = ps.tile(
                [128, NT, F], mybir.dt.float32, padded_shape=[128, NT, 512]
            )

            CW = TPC * F  # chunk width in x/o columns (256)
            for c in range(NCHUNK):
                for t in range(c * TPC, (c + 1) * TPC):
                    i, h = t // 2, t % 2
                    nc.tensor.matmul(
                        y_ps[:, t],
                        in_sb[h * F : (h + 1) * F, XT0 + i * 128 : XT0 + (i + 1) * 128],
                        in_sb[h * F : (h + 1) * F, 0:F],
                        start=True,
                        stop=True,
                    )

                yc = y_ps[:, c * TPC : (c + 1) * TPC]          # [128, TPC, F]
                xc = in_sb[:, X0 + c * CW : X0 + (c + 1) * CW].rearrange(
                    "p (t c) -> p t c", t=TPC
                )
                oc = o_sb[:, c * CW : (c + 1) * CW].rearrange(
                    "p (t c) -> p t c", t=TPC
                )

                u = ew.tile([128, CW], mybir.dt.float32)
                nc.scalar.activation(
                    u[:].rearrange("p (t c) -> p t c", t=TPC), yc, AF.Abs
                )
                # s2 = (y >= 0) * 2 in {2, 0}; w*sign(y) = (s2 - 1) * w
                s2 = ew.tile([128, CW], mybir.dt.float32)
                nc.vector.tensor_scalar(
                    s2[:].rearrange("p (t c) -> p t c", t=TPC),
                    yc, 0.0, 2.0, ALU.is_ge, ALU.mult,
                )
                e = ew.tile([128, CW], mybir.dt.float32)
                nc.scalar.activation(e[:], u[:], AF.Exp, scale=-1.0)
                w = ew.tile([128, CW], mybir.dt.float32)
                nc.scalar.activation(w[:], e[:], AF.Ln, bias=half[:], scale=0.5)
                p = ew.tile([128, CW], mybir.dt.float32)
                nc.vector.tensor_tensor(
                    p[:].rearrange("p (t c) -> p t c", t=TPC), yc, xc, ALU.add
                )
                q = ew.tile([128, CW], mybir.dt.float32)
                nc.vector.scalar_tensor_tensor(
                    q[:], s2[:], -1.0, w[:], ALU.add, ALU.mult
                )
                nc.vector.tensor_tensor(oc, p[:].rearrange("p (t c) -> p t c", t=TPC), q[:].rearrange("p (t c) -> p t c", t=TPC), ALU.add)
                eng = nc.sync if c % 2 == 0 else nc.scalar
                eng.dma_start(
                    o_d[:, c * CW : (c + 1) * CW], o_sb[:, c * CW : (c + 1) * CW]
                )
    nc.compile()
    return nc


def _prep_core_inputs(x, Wv):
    """Pack each core's inputs into one [128, 1088] tensor: [wv2 | xt | xp].

    wv2: Wv stacked twice (h0/h64 matmul pairs need rhs at both base
    partitions).  xt[h*64+ch, 128*i + p] = xs[(2i+h)*128 + p, ch].
    xp[p, t*64+c] = xs[t*128 + p, c] (the SBUF node-tile layout).
    """
    wv2 = np.vstack([Wv, Wv])  # [128, F]
    maps = []
    for i in range(N_CORES):
        xs = x[i * NS : (i + 1) * NS]
        xt = xs.reshape(NT // 2, 2, 128, F).transpose(1, 3, 0, 2).reshape(128, NT * F)
        xp = xs.reshape(NT, 128, F).transpose(1, 0, 2).reshape(128, NT * F)
        maps.append(
            {"in_pack": np.ascontiguousarray(np.hstack([wv2, xt, xp]))}
        )
    return maps


def _run(x, Wv, edges_dst, trace=False):
    x = np.ascontiguousarray(np.asarray(x, dtype=np.float32))
    Wv = np.ascontiguousarray(np.asarray(Wv, dtype=np.float32))
    if "nc" not in _cache:
        _cache["nc"] = _build_bass()
    nc = _cache["nc"]
    in_maps = _prep_core_inputs(x, Wv)
    res = run_bass_kernel_spmd(
        nc, in_maps, core_ids=list(range(N_CORES)), trace=trace
    )
    # Unpack [128, NT*F] node-tile layout back to [NS, F] per core.
    out = np.concatenate(
        [
            r["out"].reshape(128, NT, F).transpose(1, 0, 2).reshape(NS, F)
            for r in res.results
        ],
        axis=0,
    )
    # Residual-only rows: nodes with no incoming edge keep x unchanged.
    indeg = np.bincount(
        np.asarray(edges_dst).astype(np.int64), minlength=N_NODES
    )
    dead = indeg == 0
    if dead.any():
        out[dead] = x[dead]
    return out, res


def kernel(x, Wq, Wk, Wv, Wqk, edges_src, edges_dst):
    out, _ = _run(x, Wv, edges_dst)
    return out
